# revision 11
# baseline (speedup 1.0000x reference)
"""nn_ContactHead Trainium2 kernel (8-core data parallel).

Math: out = sigmoid(w2 . relu((grid_sample(feat, uv) @ reduce_w + reduce_b) @ cls_w1 + cls_b1) + cls_b2)

Restructure (everything left of the relu is linear, and bilinear sampling is
linear in the features, so it commutes):
  W  = reduce_w @ cls_w1            (1280 x 128)   [computed on device]
  bb = reduce_b @ cls_w1 + cls_b1   (128)          [computed on device]
  z[d, pix]   = feat[:, pix] . W[:, d] + bb[d]     at all 1024 pixels (PE, fp32r)
  bilinear via pre-differenced quantities (single gather index per vert):
    z00 = z;  dzx = z(x+1)-z;  dzy = z(y+1)-z;  dzxy = dzy(x+1)-dzy
    v(wx,wy) = z00 + wx*dzx + wy*(dzy + wx*dzxy)
  gather of the 4 quantities on GPSIMD ap_gather (bf16 pairs packed in uint32,
  2 images stacked across the 128 partitions -> 64 dim-pairs per image).
  blend on DVE with broadcast-AP weights, dot+sigmoid via PE/ACT.

Layout bijection (device blend column for compute slot (p,q), p=16g+r):
  f(p, q) = 864*g + 16*q + r      host unpermutes the output with this.
"""

import ml_dtypes
import numpy as np

B, C, H, W, N = 32, 1280, 32, 32, 6890
NCORES = 8
IMGS = B // NCORES          # 4 images per core
PIX = H * W                 # 1024
PIXPAD = 1152               # padded pixel slots (uint32 units) in z tiles
NCH = C // 128              # 10 channel chunks
MID = 128                   # final feature dim
NV = 6912                   # padded verts  (= 54*128 = 4*1728)
Q = NV // 128               # 54 columns in the [128, 54] prep layout
VCH = 4                     # vert chunks per stack
VCN = NV // VCH             # 1728 verts per chunk
NBLK = 4                    # dot blocks per vert chunk
BLK = VCN // NBLK           # 432

_CACHE = {}


def _f(p, q):
    g, r = p // 16, p % 16
    return 864 * g + 16 * q + r


def _build():
    if "nc" in _CACHE:
        return _CACHE["nc"]

    from contextlib import ExitStack

    import concourse.bass as bass
    import concourse.tile as tile
    from concourse import bacc, mybir
    from concourse.ap import AP

    f32 = mybir.dt.float32
    f32r = mybir.dt.float32r
    bf16 = mybir.dt.bfloat16
    i16 = mybir.dt.int16
    u32 = mybir.dt.uint32
    OP = mybir.AluOpType
    ACT = mybir.ActivationFunctionType

    nc = bacc.Bacc("TRN2", target_bir_lowering=False, debug=False)

    feat_d = nc.dram_tensor("feat", [IMGS, C, PIX], bf16, kind="ExternalInput")
    uv_d = nc.dram_tensor("uv", [IMGS, NV, 2], f32, kind="ExternalInput")
    rwt_d = nc.dram_tensor("rwt", [256, C], f32, kind="ExternalInput")
    cw1_d = nc.dram_tensor("cw1", [256, MID], f32, kind="ExternalInput")
    rb_d = nc.dram_tensor("rb", [256], f32, kind="ExternalInput")
    cb1_d = nc.dram_tensor("cb1", [MID], f32, kind="ExternalInput")
    w2c_d = nc.dram_tensor("w2c", [128, 4], f32, kind="ExternalInput")
    cb2_d = nc.dram_tensor("cb2", [2, 1], f32, kind="ExternalInput")
    out_d = nc.dram_tensor("out", [IMGS, NV], f32, kind="ExternalOutput")

    with tile.TileContext(nc) as tc, ExitStack() as ctx:
        consts = ctx.enter_context(tc.tile_pool(name="consts", bufs=1))
        prep = ctx.enter_context(tc.tile_pool(name="prep", bufs=1))
        featp = ctx.enter_context(tc.tile_pool(name="featp", bufs=3))
        zqp = ctx.enter_context(tc.tile_pool(name="zqp", bufs=8))
        wrp = ctx.enter_context(tc.tile_pool(name="wrp", bufs=2))
        irp = ctx.enter_context(tc.tile_pool(name="irp", bufs=2))
        gp = ctx.enter_context(tc.tile_pool(name="gp", bufs=6))
        tp = ctx.enter_context(tc.tile_pool(name="tp", bufs=2))
        sm = ctx.enter_context(tc.tile_pool(name="sm", bufs=4))
        obp = ctx.enter_context(tc.tile_pool(name="obp", bufs=4))

        # ---------------- phase 0: combined weights ----------------
        psw_ctx = ExitStack()
        psw = psw_ctx.enter_context(tc.tile_pool(name="psw", bufs=2, space="PSUM"))
        rwt_t = []
        cw1_t = []
        for k in range(2):
            rt = prep.tile([128, C], f32, tag=f"rwt{k}", name=f"rwt{k}")
            nc.sync.dma_start(rt[:], rwt_d.ap()[128 * k : 128 * (k + 1), :])
            rwt_t.append(rt)
            ct = consts.tile([128, MID], f32, tag=f"cw1{k}", name=f"cw1{k}")
            nc.sync.dma_start(ct[:], cw1_d.ap()[128 * k : 128 * (k + 1), :])
            cw1_t.append(ct)

        W_e, W_o = [], []
        for c in range(NCH):
            pw = psw.tile([128, 128], f32, tag="pw", name=f"pw{c}")
            for k in range(2):
                nc.tensor.matmul(
                    pw[:],
                    lhsT=rwt_t[k][:, 128 * c : 128 * (c + 1)],
                    rhs=cw1_t[k][:],
                    start=(k == 0),
                    stop=(k == 1),
                )
            we = consts.tile([128, 64], bf16, tag=f"we{c}", name=f"we{c}")
            wo = consts.tile([128, 64], bf16, tag=f"wo{c}", name=f"wo{c}")
            nc.scalar.copy(we[:], pw[:, 0:128:2])
            nc.scalar.copy(wo[:], pw[:, 1:128:2])
            W_e.append(we)
            W_o.append(wo)

        rb_t = prep.tile([128, 2], f32, tag="rb", name="rb")
        nc.sync.dma_start(rb_t[:], rb_d.ap().rearrange("(k p) -> p k", p=128))
        cb1_t = prep.tile([1, MID], f32, tag="cb1", name="cb1")
        nc.sync.dma_start(cb1_t[:], cb1_d.ap().rearrange("(one d) -> one d", one=1))
        pb = psw.tile([1, 128], f32, tag="pb", name="pb")
        for k in range(2):
            nc.tensor.matmul(
                pb[:],
                lhsT=rb_t[:, k : k + 1],
                rhs=cw1_t[k][:],
                start=(k == 0),
                stop=(k == 1),
            )
        brow = prep.tile([1, 128], f32, tag="brow", name="brow")
        nc.vector.tensor_tensor(out=brow[:], in0=pb[:], in1=cb1_t[:], op=OP.add)
        b_e = consts.tile([1, 64], bf16, tag="b_e", name="b_e")
        b_o = consts.tile([1, 64], bf16, tag="b_o", name="b_o")
        nc.scalar.copy(b_e[:], brow[:, 0:128:2])
        nc.scalar.copy(b_o[:], brow[:, 1:128:2])

        ones_t = consts.tile([1, PIX], bf16, tag="ones", name="ones")
        nc.vector.memset(ones_t[:], 1.0)

        w2cf = prep.tile([128, 4], f32, tag="w2cf", name="w2cf")
        nc.sync.dma_start(w2cf[:], w2c_d.ap())
        w2cb = consts.tile([128, 4], bf16, tag="w2cb", name="w2cb")
        nc.vector.tensor_copy(out=w2cb[:], in_=w2cf[:])
        cb2_t = consts.tile([2, 1], f32, tag="cb2", name="cb2")
        nc.sync.dma_start(cb2_t[:], cb2_d.ap())

        psw_ctx.close()
        zps = ctx.enter_context(tc.tile_pool(name="zps", bufs=3, space="PSUM"))
        psl = ctx.enter_context(tc.tile_pool(name="psl", bufs=2, space="PSUM"))

        # ---------------- per-image uv prep ----------------
        # produces, per image: wrapped idx rows + flat packed (wx, wy) weights
        # written directly into the per-stack replication tiles.
        wrep = []   # [stack] -> [128, 2*NV] bf16   (wx,wy packed per vert)
        idxrep = []  # [stack] -> [128, 8*Q] i16
        for s in range(2):
            wr = wrp.tile([128, 2 * NV], bf16, tag="wrep", name=f"wrep{s}")
            ir = irp.tile([128, 8 * Q], i16, tag="idxrep", name=f"idxrep{s}")
            wrep.append(wr)
            idxrep.append(ir)

        def emit_floor(dst, srcap, pool, nm):
            """dst = floor(srcap) for srcap in [0, 32); robust to convert rounding mode."""
            ti = pool.tile([128, Q], i16, tag="flt_i", name=f"fi_{nm}")
            tf = pool.tile([128, Q], f32, tag="flt_f", name=f"ff_{nm}")
            nc.vector.tensor_copy(out=ti[:], in_=srcap)
            nc.vector.tensor_copy(out=dst, in_=ti[:])
            nc.vector.tensor_tensor(out=tf[:], in0=dst, in1=srcap, op=OP.is_gt)
            nc.vector.tensor_tensor(out=dst, in0=dst, in1=tf[:], op=OP.subtract)

        for i in range(IMGS):
            s, h = i // 2, i % 2
            # ---- weight path: block layout (vert j at (j//54, j%54)) ----
            uvw = sm.tile([128, 2 * Q], f32, tag="uvw", name=f"uvw{i}")
            nc.sync.dma_start(
                uvw[:], uv_d.ap()[i].rearrange("(p q) two -> p (q two)", p=128)
            )
            pxw = sm.tile([128, Q], f32, tag="pxw", name=f"pxw{i}")
            pyw = sm.tile([128, Q], f32, tag="pyw", name=f"pyw{i}")
            nc.vector.tensor_scalar(out=pxw[:], in0=uvw[:, 0 : 2 * Q : 2],
                                    scalar1=15.5, scalar2=15.5, op0=OP.mult, op1=OP.add)
            nc.vector.tensor_scalar(out=pyw[:], in0=uvw[:, 1 : 2 * Q : 2],
                                    scalar1=15.5, scalar2=15.5, op0=OP.mult, op1=OP.add)
            x0w = sm.tile([128, Q], f32, tag="x0w", name=f"x0w{i}")
            y0w = sm.tile([128, Q], f32, tag="y0w", name=f"y0w{i}")
            emit_floor(x0w[:], pxw[:], sm, f"xw{i}")
            emit_floor(y0w[:], pyw[:], sm, f"yw{i}")
            nc.vector.tensor_scalar(out=x0w[:], in0=x0w[:], scalar1=30.0, scalar2=0.0,
                                    op0=OP.min, op1=OP.max)
            nc.vector.tensor_scalar(out=y0w[:], in0=y0w[:], scalar1=30.0, scalar2=0.0,
                                    op0=OP.min, op1=OP.max)
            # wx = px - x0 ; wy = py - y0, packed (wx, wy) as bf16 pairs
            wxw = sm.tile([128, Q], f32, tag="wxw", name=f"wxw{i}")
            wyw = sm.tile([128, Q], f32, tag="wyw", name=f"wyw{i}")
            nc.vector.tensor_tensor(out=wxw[:], in0=pxw[:], in1=x0w[:], op=OP.subtract)
            nc.vector.tensor_tensor(out=wyw[:], in0=pyw[:], in1=y0w[:], op=OP.subtract)
            wpk = sm.tile([128, 2 * Q], bf16, tag="wpk", name=f"wpk{i}")
            nc.vector.tensor_copy(out=wpk[:, 0 : 2 * Q : 2], in_=wxw[:])
            nc.vector.tensor_copy(out=wpk[:, 1 : 2 * Q : 2], in_=wyw[:])
            # flat write: wrep row 64h gets (wx,wy)[vert j] at u32 position j
            wr_u32 = wrep[s][:].bitcast(u32)   # [128, NV]
            dst = AP(
                wr_u32.tensor,
                wr_u32.offset + 64 * h * wr_u32.ap[0][0],
                [[wr_u32.ap[0][0], 1], [1, NV]],
            )
            nc.sync.dma_start(dst, wpk[:].bitcast(u32))

            # ---- idx path: f-layout (slot (p,q), p=16g+r holds vert 864g+16q+r) ----
            uvi = sm.tile([128, 2 * Q], f32, tag="uvi", name=f"uvi{i}")
            uvr = uv_d.ap()[i]  # (NV, 2)
            for g in range(8):
                srcg = AP(
                    uvr.tensor,
                    uvr.offset + 2 * 864 * g * uvr.ap[-1][0],
                    [[2 * uvr.ap[-1][0], 16], [32 * uvr.ap[-1][0], Q],
                     [uvr.ap[-1][0], 2]],
                )
                nc.sync.dma_start(uvi[16 * g : 16 * (g + 1), :], srcg)
            pxi = sm.tile([128, Q], f32, tag="pxi", name=f"pxi{i}")
            pyi = sm.tile([128, Q], f32, tag="pyi", name=f"pyi{i}")
            nc.vector.tensor_scalar(out=pxi[:], in0=uvi[:, 0 : 2 * Q : 2],
                                    scalar1=15.5, scalar2=15.5, op0=OP.mult, op1=OP.add)
            nc.vector.tensor_scalar(out=pyi[:], in0=uvi[:, 1 : 2 * Q : 2],
                                    scalar1=15.5, scalar2=15.5, op0=OP.mult, op1=OP.add)
            x0i = sm.tile([128, Q], f32, tag="x0i", name=f"x0i{i}")
            y0i = sm.tile([128, Q], f32, tag="y0i", name=f"y0i{i}")
            emit_floor(x0i[:], pxi[:], sm, f"xi{i}")
            emit_floor(y0i[:], pyi[:], sm, f"yi{i}")
            pxi, pyi = x0i, y0i
            nc.vector.tensor_scalar(out=pxi[:], in0=pxi[:], scalar1=30.0, scalar2=0.0,
                                    op0=OP.min, op1=OP.max)
            nc.vector.tensor_scalar(out=pyi[:], in0=pyi[:], scalar1=30.0, scalar2=0.0,
                                    op0=OP.min, op1=OP.max)
            idxf = sm.tile([128, Q], f32, tag="idxf", name=f"idxf{i}")
            nc.vector.scalar_tensor_tensor(
                out=idxf[:], in0=pyi[:], scalar=32.0, in1=pxi[:],
                op0=OP.mult, op1=OP.add,
            )
            idxi = sm.tile([128, Q], i16, tag="idxi", name=f"idxi{i}")
            nc.vector.tensor_copy(out=idxi[:], in_=idxf[:])
            # wrapped idx: slot j at (j%16, j//16); vert at (16g+r, q) is j=864g+16q+r
            # -> wrapped (r, 54g+q): 8 affine DMAs
            ir_ap = idxrep[s][:]
            for g in range(8):
                idst = AP(
                    ir_ap.tensor,
                    ir_ap.offset + 64 * h * ir_ap.ap[0][0] + Q * g * ir_ap.ap[-1][0],
                    [[ir_ap.ap[0][0], 16], [1, Q]],
                )
                nc.sync.dma_start(idst, idxi[16 * g : 16 * (g + 1), :])

            # replicate within the 64-partition half: weights 64h -> 64h+64
            for dbl in range(6):
                n = 1 << dbl
                nc.sync.dma_start(
                    wrep[s][64 * h + n : 64 * h + 2 * n, :],
                    wrep[s][64 * h : 64 * h + n, :],
                )
            # idx: rows [64h:64h+16) -> fill [64h:64h+64)
            for dbl in range(2):
                n = 16 << dbl
                nc.sync.dma_start(
                    idxrep[s][64 * h + n : 64 * h + 2 * n, :],
                    idxrep[s][64 * h : 64 * h + n, :],
                )

        # ---------------- per-stack main pipeline ----------------
        for s in range(2):
            zpe = zps.tile([128, PIX], f32, tag="zp", name=f"zpe{s}")
            zpo = zps.tile([128, PIX], f32, tag="zp", name=f"zpo{s}")
            for h in range(2):
                i = 2 * s + h
                for c in range(NCH):
                    ft = featp.tile([128, PIX], bf16, tag="ft", name=f"ft{i}_{c}")
                    nc.sync.dma_start(
                        ft[:], feat_d.ap()[i, 128 * c : 128 * (c + 1), :]
                    )
                    for ph in range(2):
                        for zp, Wt in ((zpe, W_e[c]), (zpo, W_o[c])):
                            nc.tensor.matmul(
                                zp[64 * h : 64 * h + 64, 512 * ph : 512 * (ph + 1)],
                                lhsT=Wt[:],
                                rhs=ft[:, 512 * ph : 512 * (ph + 1)],
                                start=(c == 0),
                                stop=False,
                                skip_group_check=True,
                            )
                for ph in range(2):
                    for zp, bt in ((zpe, b_e), (zpo, b_o)):
                        nc.tensor.matmul(
                            zp[64 * h : 64 * h + 64, 512 * ph : 512 * (ph + 1)],
                            lhsT=bt[:],
                            rhs=ones_t[:, 512 * ph : 512 * (ph + 1)],
                            start=False,
                            stop=True,
                            skip_group_check=True,
                        )

            # escape to packed bf16 (pairs of dims in one uint32 slot)
            zq = zqp.tile([128, 2 * PIXPAD], bf16, tag="zq", name=f"zq{s}")
            dzx = zqp.tile([128, 2 * PIXPAD], bf16, tag="zq", name=f"dzx{s}")
            dzy = zqp.tile([128, 2 * PIXPAD], bf16, tag="zq", name=f"dzy{s}")
            dzxy = zqp.tile([128, 2 * PIXPAD], bf16, tag="zq", name=f"dzxy{s}")
            for h in range(2):
                nc.scalar.copy(
                    zq[64 * h : 64 * h + 64, 0 : 2 * PIX : 2],
                    zpe[64 * h : 64 * h + 64, :],
                )
                nc.scalar.copy(
                    zq[64 * h : 64 * h + 64, 1 : 2 * PIX : 2],
                    zpo[64 * h : 64 * h + 64, :],
                )
            nc.vector.memset(zq[:, 2 * PIX : 2 * PIXPAD], 0.0)
            # pre-differenced quantities (slot-preserving shifts)
            nc.vector.tensor_tensor(
                out=dzx[:, 0:2240], in0=zq[:, 2:2242], in1=zq[:, 0:2240],
                op=OP.subtract,
            )
            nc.vector.tensor_tensor(
                out=dzy[:, 0:2240], in0=zq[:, 64:2304], in1=zq[:, 0:2240],
                op=OP.subtract,
            )
            nc.vector.tensor_tensor(
                out=dzxy[:, 0:2176], in0=dzy[:, 2:2178], in1=dzy[:, 0:2176],
                op=OP.subtract,
            )
            nc.vector.memset(dzx[:, 2240 : 2 * PIXPAD], 0.0)
            nc.vector.memset(dzy[:, 2240 : 2 * PIXPAD], 0.0)
            nc.vector.memset(dzxy[:, 2176 : 2 * PIXPAD], 0.0)

            # vert chunks: gather + blend + dot + sigmoid + out
            for v in range(VCH):
                idx_ap = idxrep[s][:, 2 * Q * v : 2 * Q * (v + 1)]
                gts = {}
                for nm, zt in (("g0", zq), ("gx", dzx), ("gy", dzy), ("gxy", dzxy)):
                    gt = gp.tile([128, 2 * VCN], bf16, tag="g", name=f"{nm}_{s}_{v}")
                    nc.gpsimd.ap_gather(
                        out_ap=gt[:].bitcast(u32),
                        in_ap=zt[:].bitcast(u32),
                        idxs_ap=idx_ap,
                        channels=128,
                        num_elems=PIXPAD,
                        d=1,
                        num_idxs=VCN,
                    )
                    gts[nm] = gt

                wr_bf = wrep[s][:]

                def wap(off):
                    # [128, VCN, 2] with the pair-duplicated step-0 inner dim
                    return AP(
                        wr_bf.tensor,
                        wr_bf.offset + (2 * VCN * v + off) * wr_bf.ap[-1][0],
                        [[wr_bf.ap[0][0], 128], [2, VCN], [0, 2]],
                    )

                def t3(t):
                    return t[:].rearrange("p (n two) -> p n two", two=2)

                g0, gx, gy, gxy = gts["g0"], gts["gx"], gts["gy"], gts["gxy"]
                t1 = tp.tile([128, 2 * VCN], bf16, tag="t1", name=f"t1_{s}_{v}")
                # t1 = wx*dzx ; g0 += t1
                nc.vector.tensor_tensor(out=t3(t1), in0=t3(gx), in1=wap(0), op=OP.mult)
                nc.vector.tensor_tensor(out=g0[:], in0=g0[:], in1=t1[:], op=OP.add)
                # t1 = wx*dzxy ; gy += t1
                nc.vector.tensor_tensor(out=t3(t1), in0=t3(gxy), in1=wap(0), op=OP.mult)
                nc.vector.tensor_tensor(out=gy[:], in0=gy[:], in1=t1[:], op=OP.add)
                # t1 = wy*gy ; v = g0 + t1
                nc.vector.tensor_tensor(out=t3(t1), in0=t3(gy), in1=wap(1), op=OP.mult)
                nc.vector.tensor_tensor(out=g0[:], in0=g0[:], in1=t1[:], op=OP.add)
                # relu in place
                nc.scalar.activation(g0[:], g0[:], ACT.Relu)

                for blk in range(NBLK):
                    pl = psl.tile([2, BLK], f32, tag="pl", name=f"pl{s}_{v}_{blk}")
                    base = 2 * BLK * blk
                    rhs_e = AP(
                        g0[:].tensor,
                        g0[:].offset + base * g0[:].ap[-1][0],
                        [[g0[:].ap[0][0], 128], [2, BLK]],
                    )
                    rhs_o = AP(
                        g0[:].tensor,
                        g0[:].offset + (base + 1) * g0[:].ap[-1][0],
                        [[g0[:].ap[0][0], 128], [2, BLK]],
                    )
                    nc.tensor.matmul(pl[:], lhsT=w2cb[:, 0:2], rhs=rhs_e,
                                     start=True, stop=False)
                    nc.tensor.matmul(pl[:], lhsT=w2cb[:, 2:4], rhs=rhs_o,
                                     start=False, stop=True)
                    ob = obp.tile([2, BLK], f32, tag="ob", name=f"ob{s}_{v}_{blk}")
                    nc.scalar.activation(ob[:], pl[:], ACT.Sigmoid, bias=cb2_t[:])
                    nc.sync.dma_start(
                        out_d.ap()[2 * s : 2 * s + 2, VCN * v + BLK * blk :
                                   VCN * v + BLK * (blk + 1)],
                        ob[:],
                    )

    nc.compile()
    _CACHE["nc"] = nc
    return nc


def _host_prep(inputs):
    feat = np.asarray(inputs["feat_map"], dtype=np.float32)
    uv = np.asarray(inputs["verts_uv"], dtype=np.float32)
    rw = np.asarray(inputs["reduce_w"], dtype=np.float32)
    rb = np.asarray(inputs["reduce_b"], dtype=np.float32)
    w1 = np.asarray(inputs["cls_w1"], dtype=np.float32)
    b1 = np.asarray(inputs["cls_b1"], dtype=np.float32)
    w2 = np.asarray(inputs["cls_w2"], dtype=np.float32)
    b2 = np.asarray(inputs["cls_b2"], dtype=np.float32)

    rwt = np.ascontiguousarray(rw.T)                      # (256, 1280)
    uvp = np.zeros((B, NV, 2), dtype=np.float32)
    uvp[:, :N, :] = uv
    featr = feat.reshape(B, C, PIX)

    w2c = np.zeros((128, 4), dtype=np.float32)
    w2c[:64, 0] = w2[0::2]
    w2c[64:, 1] = w2[0::2]
    w2c[:64, 2] = w2[1::2]
    w2c[64:, 3] = w2[1::2]
    cb2 = np.full((2, 1), b2[0], dtype=np.float32)

    shared = {
        "rwt": rwt,
        "cw1": np.ascontiguousarray(w1),
        "rb": rb,
        "cb1": b1,
        "w2c": w2c,
        "cb2": cb2,
    }
    in_maps = []
    for core in range(NCORES):
        sl = slice(core * IMGS, (core + 1) * IMGS)
        m = dict(shared)
        m["feat"] = np.ascontiguousarray(featr[sl]).astype(ml_dtypes.bfloat16)
        m["uv"] = np.ascontiguousarray(uvp[sl])
        in_maps.append(m)
    return in_maps


_PERM = None


def _out_perm():
    global _PERM
    if _PERM is None:
        perm = np.empty(NV, dtype=np.int64)
        for p in range(128):
            for q in range(Q):
                perm[_f(p, q)] = 54 * p + q
        # inverse: out_orig[perm[dv]] = dev[dv]
        _PERM = perm
    return _PERM


def kernel(**inputs):
    from concourse.bass_utils import run_bass_kernel_spmd

    nc = _build()
    in_maps = _host_prep(inputs)
    res = run_bass_kernel_spmd(nc, in_maps, list(range(NCORES)))
    out = np.empty((B, N), dtype=np.float32)
    for core in range(NCORES):
        dev = res.results[core]["out"]          # (IMGS, NV), vert order = natural
        out[core * IMGS : (core + 1) * IMGS] = dev[:, :N]
    return out


# revision 18
# speedup vs baseline: 3.1234x; 3.1234x over previous
"""nn_ContactHead Trainium2 kernel (8-core data parallel).

out = sigmoid(w2 . relu((grid_sample(feat, uv) @ reduce_w + reduce_b) @ cls_w1 + cls_b1) + cls_b2)

Everything left of the relu is linear and bilinear sampling is linear in the
features, so the channel reductions commute with the sampling:
  W  = reduce_w @ cls_w1            (1280 x 128)   [device, PE]
  bb = reduce_b @ cls_w1 + cls_b1   (128)          [device, PE via ones-row]
  z[d, pix] = feat[:, pix].W[:, d] + bb[d]    at the 1024 pixels (PE, bf16)
Bilinear via pre-differenced pixel quantities (one gather row per vert):
  dzx = z(x+1)-z ; dzy = z(y+1)-z ; dzxy = dzy(x+1)-dzy
  v(wx,wy) = z00 + wx*dzx + wy*(dzy + wx*dzxy)
Tokens [z00|dzx|dzy|dzxy] (1KB bf16 rows, pixel-major) are written to DRAM
(PE transpose), then fetched per-vert with the hardware DMA gather
(non-transpose => verts land on partitions, 128 dims x 4 quantities on free).
Blend on DVE with free-dim step-0 broadcast weight APs, relu+w2 fused via
scalar_tensor_tensor, dot via tensor_reduce, sigmoid on ACT.

Vert layout: vert j lives at (partition j%128, column j//128).
"""

import ml_dtypes
import numpy as np

B, C, H, W, N = 32, 1280, 32, 32, 6890
NCORES = 8
IMGS = B // NCORES          # 4 images per core
PIX = H * W                 # 1024
PPAD = 1088                 # padded pixel slots in the dims-major z tiles
NCH = C // 128              # 10 channel chunks
MID = 128
NV = 6912                   # padded verts (= 54*128)
Q = NV // 128               # 54
VCH = 6                     # vert chunks per image
VCN = NV // VCH             # 1152 = 9*128
VROW = VCN // 128           # 9 rows per gathered chunk tile
TOK = 512                   # token row: 4 quantities x 128 dims (bf16)

_CACHE = {}


def _build():
    if "nc" in _CACHE:
        return _CACHE["nc"]

    from contextlib import ExitStack

    import concourse.bass as bass
    import concourse.tile as tile
    from concourse import bacc, mybir
    from concourse.ap import AP
    from concourse.bass import IndirectOffsetOnAxis

    f32 = mybir.dt.float32
    bf16 = mybir.dt.bfloat16
    i16 = mybir.dt.int16
    i32 = mybir.dt.int32
    OP = mybir.AluOpType
    ACT = mybir.ActivationFunctionType

    nc = bacc.Bacc("TRN2", target_bir_lowering=False, debug=False)

    feat_d = nc.dram_tensor("feat", [IMGS, C, PIX], bf16, kind="ExternalInput")
    uv_d = nc.dram_tensor("uv", [IMGS, NV, 2], f32, kind="ExternalInput")
    rwt_d = nc.dram_tensor("rwt", [256, C], f32, kind="ExternalInput")
    cw1_d = nc.dram_tensor("cw1", [256, MID], f32, kind="ExternalInput")
    rb_d = nc.dram_tensor("rb", [256], f32, kind="ExternalInput")
    cb1_d = nc.dram_tensor("cb1", [MID], f32, kind="ExternalInput")
    w2r_d = nc.dram_tensor("w2r", [128, 128], f32, kind="ExternalInput")
    cb2_d = nc.dram_tensor("cb2", [128, 1], f32, kind="ExternalInput")
    id_d = nc.dram_tensor("ident", [128, 128], bf16, kind="ExternalInput")
    ztok_d = [
        nc.dram_tensor(f"ztok{i}", [PIX, TOK], bf16) for i in range(IMGS)
    ]
    out_d = nc.dram_tensor("out", [IMGS, NV], f32, kind="ExternalOutput")

    with tile.TileContext(nc) as tc, ExitStack() as ctx:
        consts = ctx.enter_context(tc.tile_pool(name="consts", bufs=1))
        prep = ctx.enter_context(tc.tile_pool(name="prep", bufs=1))
        featp = ctx.enter_context(tc.tile_pool(name="featp", bufs=2))
        zqp = ctx.enter_context(tc.tile_pool(name="zqp", bufs=8))
        gpool = ctx.enter_context(tc.tile_pool(name="gpool", bufs=4))
        tpool = ctx.enter_context(tc.tile_pool(name="tpool", bufs=4))
        sm = ctx.enter_context(tc.tile_pool(name="sm", bufs=4))
        irp = ctx.enter_context(tc.tile_pool(name="irp", bufs=4))
        lg = ctx.enter_context(tc.tile_pool(name="lg", bufs=2))

        # ---------------- phase 0: combined weights (PE) ----------------
        psw_ctx = ExitStack()
        psw = psw_ctx.enter_context(tc.tile_pool(name="psw", bufs=2, space="PSUM"))
        rwt_t, cw1_t = [], []
        for k in range(2):
            rt = prep.tile([128, C], f32, tag=f"rwt{k}", name=f"rwt{k}")
            nc.sync.dma_start(rt[:], rwt_d.ap()[128 * k : 128 * (k + 1), :])
            rwt_t.append(rt)
            ct = prep.tile([128, MID], f32, tag=f"cw1{k}", name=f"cw1{k}")
            nc.sync.dma_start(ct[:], cw1_d.ap()[128 * k : 128 * (k + 1), :])
            cw1_t.append(ct)

        Wt = []
        for c in range(NCH):
            pw = psw.tile([128, 128], f32, tag="pw", name=f"pw{c}")
            for k in range(2):
                nc.tensor.matmul(
                    pw[:],
                    lhsT=rwt_t[k][:, 128 * c : 128 * (c + 1)],
                    rhs=cw1_t[k][:],
                    start=(k == 0),
                    stop=(k == 1),
                )
            wt = consts.tile([128, 128], bf16, tag=f"W{c}", name=f"W{c}")
            nc.scalar.copy(wt[:], pw[:])
            Wt.append(wt)

        rb_t = prep.tile([128, 2], f32, tag="rb", name="rb")
        nc.scalar.dma_start(rb_t[:], rb_d.ap().rearrange("(k p) -> p k", p=128))
        cb1_t = prep.tile([1, MID], f32, tag="cb1", name="cb1")
        nc.scalar.dma_start(cb1_t[:], cb1_d.ap().rearrange("(one d) -> one d", one=1))
        pb = psw.tile([1, 128], f32, tag="pb", name="pb")
        for k in range(2):
            nc.tensor.matmul(
                pb[:], lhsT=rb_t[:, k : k + 1], rhs=cw1_t[k][:],
                start=(k == 0), stop=(k == 1),
            )
        brow = prep.tile([1, 128], f32, tag="brow", name="brow")
        nc.vector.tensor_tensor(out=brow[:], in0=pb[:], in1=cb1_t[:], op=OP.add)
        bbias = consts.tile([1, 128], bf16, tag="bbias", name="bbias")
        nc.scalar.copy(bbias[:], brow[:])

        ones_t = consts.tile([1, PIX], bf16, tag="ones", name="ones")
        nc.vector.memset(ones_t[:], 1.0)
        ident = consts.tile([128, 128], bf16, tag="ident", name="ident")
        nc.scalar.dma_start(ident[:], id_d.ap())
        w2rf = prep.tile([128, 128], f32, tag="w2rf", name="w2rf")
        nc.scalar.dma_start(w2rf[:], w2r_d.ap())
        w2rep = consts.tile([128, 128], bf16, tag="w2rep", name="w2rep")
        nc.vector.tensor_copy(out=w2rep[:], in_=w2rf[:])
        cb2_t = consts.tile([128, 1], f32, tag="cb2", name="cb2")
        nc.scalar.dma_start(cb2_t[:], cb2_d.ap())
        psw_ctx.close()

        zps = ctx.enter_context(tc.tile_pool(name="zps", bufs=2, space="PSUM"))
        pst = ctx.enter_context(tc.tile_pool(name="pst", bufs=3, space="PSUM"))

        def emit_floor(dst, srcap, nm):
            """dst = floor(srcap), srcap in [0, 32); robust to convert rounding."""
            ti = sm.tile([128, Q], i16, tag="flt_i", name=f"fi_{nm}")
            tf = sm.tile([128, Q], f32, tag="flt_f", name=f"ff_{nm}")
            nc.vector.tensor_copy(out=ti[:], in_=srcap)
            nc.vector.tensor_copy(out=dst, in_=ti[:])
            nc.vector.tensor_tensor(out=tf[:], in0=dst, in1=srcap, op=OP.is_gt)
            nc.vector.tensor_tensor(out=dst, in0=dst, in1=tf[:], op=OP.subtract)

        for i in range(IMGS):
            # ---------------- uv prep: weights + wrapped idx ----------------
            # vert j at (partition j%128, col j//128)
            uvt = sm.tile([128, 2 * Q], f32, tag="uvt", name=f"uvt{i}")
            uv_i = uv_d.ap()[i]
            nc.scalar.dma_start(
                uvt[:],
                AP(uv_i.tensor, uv_i.offset, [[2, 128], [256, Q], [1, 2]]),
            )
            px = sm.tile([128, Q], f32, tag="px", name=f"px{i}")
            py = sm.tile([128, Q], f32, tag="py", name=f"py{i}")
            nc.vector.tensor_scalar(out=px[:], in0=uvt[:, 0 : 2 * Q : 2],
                                    scalar1=15.5, scalar2=15.5, op0=OP.mult, op1=OP.add)
            nc.vector.tensor_scalar(out=py[:], in0=uvt[:, 1 : 2 * Q : 2],
                                    scalar1=15.5, scalar2=15.5, op0=OP.mult, op1=OP.add)
            x0 = sm.tile([128, Q], f32, tag="x0", name=f"x0{i}")
            y0 = sm.tile([128, Q], f32, tag="y0", name=f"y0{i}")
            emit_floor(x0[:], px[:], f"x{i}")
            emit_floor(y0[:], py[:], f"y{i}")
            nc.vector.tensor_scalar(out=x0[:], in0=x0[:], scalar1=30.0, scalar2=0.0,
                                    op0=OP.min, op1=OP.max)
            nc.vector.tensor_scalar(out=y0[:], in0=y0[:], scalar1=30.0, scalar2=0.0,
                                    op0=OP.min, op1=OP.max)
            wxf = sm.tile([128, Q], f32, tag="wxf", name=f"wxf{i}")
            wyf = sm.tile([128, Q], f32, tag="wyf", name=f"wyf{i}")
            nc.vector.tensor_tensor(out=wxf[:], in0=px[:], in1=x0[:], op=OP.subtract)
            nc.vector.tensor_tensor(out=wyf[:], in0=py[:], in1=y0[:], op=OP.subtract)
            wx = irp.tile([128, Q], bf16, tag="wx", name=f"wx{i}")
            wy = irp.tile([128, Q], bf16, tag="wy", name=f"wy{i}")
            nc.vector.tensor_copy(out=wx[:], in_=wxf[:])
            nc.vector.tensor_copy(out=wy[:], in_=wyf[:])
            idxf = sm.tile([128, Q], f32, tag="idxf", name=f"idxf{i}")
            nc.vector.scalar_tensor_tensor(
                out=idxf[:], in0=y0[:], scalar=32.0, in1=x0[:],
                op0=OP.mult, op1=OP.add,
            )
            idxi = irp.tile([128, Q], i32, tag="idxi", name=f"idxi{i}")
            nc.vector.tensor_copy(out=idxi[:], in_=idxf[:])

            # ---------------- z at pixels (PE) ----------------
            ft = featp.tile([128, NCH * PIX], bf16, tag="ft", name=f"ft{i}")
            f_i = feat_d.ap()[i]
            nc.sync.dma_start(
                ft[:],
                AP(f_i.tensor, f_i.offset,
                   [[PIX, 128], [128 * PIX, NCH], [1, PIX]]),
            )
            zp = zps.tile([128, PIX], f32, tag="zp", name=f"zp{i}")
            for ph in range(2):
                sl = slice(512 * ph, 512 * (ph + 1))
                for c in range(NCH):
                    nc.tensor.matmul(
                        zp[:, sl],
                        lhsT=Wt[c][:],
                        rhs=ft[:, PIX * c + 512 * ph : PIX * c + 512 * (ph + 1)],
                        start=(c == 0),
                        stop=False,
                        skip_group_check=True,
                    )
                nc.tensor.matmul(
                    zp[:, sl], lhsT=bbias[:], rhs=ones_t[:, sl],
                    start=False, stop=True, skip_group_check=True,
                )

            # escape + pre-differenced quantities (dims-major, bf16)
            zq = zqp.tile([128, PPAD], bf16, tag="zq", name=f"zq{i}")
            dzx = zqp.tile([128, PPAD], bf16, tag="zq", name=f"dzx{i}")
            dzy = zqp.tile([128, PPAD], bf16, tag="zq", name=f"dzy{i}")
            dzxy = zqp.tile([128, PPAD], bf16, tag="zq", name=f"dzxy{i}")
            nc.scalar.copy(zq[:, 0:PIX], zp[:])
            nc.vector.memset(zq[:, PIX:PPAD], 0.0)
            nc.vector.tensor_tensor(out=dzx[:, 0:1056], in0=zq[:, 1:1057],
                                    in1=zq[:, 0:1056], op=OP.subtract)
            nc.vector.memset(dzx[:, 1056:PPAD], 0.0)
            nc.vector.tensor_tensor(out=dzy[:, 0:1056], in0=zq[:, 32:PPAD],
                                    in1=zq[:, 0:1056], op=OP.subtract)
            nc.vector.memset(dzy[:, 1056:PPAD], 0.0)
            nc.vector.tensor_tensor(out=dzxy[:, 0:1055], in0=dzy[:, 1:1056],
                                    in1=dzy[:, 0:1055], op=OP.subtract)
            nc.vector.memset(dzxy[:, 1055:PPAD], 0.0)

            # ---------------- tokens to DRAM (PE transpose per 128-pix block) ----
            stg = featp.tile([128, 8 * TOK], bf16, tag="stg", name=f"stg{i}")
            for b in range(8):
                pt = pst.tile([128, TOK], bf16, tag="pt", name=f"pt{i}_{b}")
                for qi, zt in enumerate((zq, dzx, dzy, dzxy)):
                    nc.tensor.transpose(
                        pt[:, 128 * qi : 128 * (qi + 1)],
                        zt[:, 128 * b : 128 * (b + 1)],
                        ident[:],
                    )
                nc.scalar.copy(stg[:, TOK * b : TOK * (b + 1)], pt[:])
            zt_i = ztok_d[i].ap()
            nc.sync.dma_start(
                AP(zt_i.tensor, zt_i.offset,
                   [[TOK, 128], [128 * TOK, 8], [1, TOK]]),
                stg[:].rearrange("p (b t) -> p b t", t=TOK),
            )

            # ---------------- gather + blend + dot per vert chunk ----------------
            logit = lg.tile([128, Q], f32, tag="logit", name=f"lg{i}")
            for ck in range(VCH):
                gt = gpool.tile([128, VROW * TOK], bf16, tag="g", name=f"g{i}_{ck}")
                g3 = gt[:].rearrange("p (r t) -> p r t", t=TOK)
                for r in range(VROW):
                    nc.gpsimd.indirect_dma_start(
                        out=g3[:, r, :],
                        out_offset=None,
                        in_=ztok_d[i].ap(),
                        in_offset=IndirectOffsetOnAxis(
                            ap=idxi[:, VROW * ck + r : VROW * ck + r + 1], axis=0
                        ),
                    )

                def wap(wtile, ck=ck):
                    a = wtile[:]
                    return AP(
                        a.tensor,
                        a.offset + VROW * ck * a.ap[-1][0],
                        [[a.ap[0][0], 128], [a.ap[-1][0], VROW], [0, 128]],
                    )

                t1 = tpool.tile([128, VROW * 128], bf16, tag="t1", name=f"t1_{i}_{ck}")
                t13 = t1[:].rearrange("p (r d) -> p r d", d=128)
                acc = tpool.tile([128, VROW * 128], bf16, tag="acc", name=f"ac{i}_{ck}")
                acc3 = acc[:].rearrange("p (r d) -> p r d", d=128)
                # t1 = wx*dzx ; acc = z00 + t1
                nc.vector.tensor_tensor(out=t13, in0=g3[:, :, 128:256], in1=wap(wx), op=OP.mult)
                nc.vector.tensor_tensor(out=acc3, in0=g3[:, :, 0:128], in1=t13, op=OP.add)
                # t1 = wx*dzxy ; t1 += dzy ; t1 *= wy ; acc += t1
                nc.vector.tensor_tensor(out=t13, in0=g3[:, :, 384:512], in1=wap(wx), op=OP.mult)
                nc.vector.tensor_tensor(out=t13, in0=g3[:, :, 256:384], in1=t13, op=OP.add)
                nc.vector.tensor_tensor(out=t13, in0=t13, in1=wap(wy), op=OP.mult)
                nc.vector.tensor_tensor(out=acc3, in0=acc3, in1=t13, op=OP.add)
                # h = relu(acc) * w2   (fused), then reduce over dims
                w2ap = AP(
                    w2rep[:].tensor, w2rep[:].offset,
                    [[w2rep[:].ap[0][0], 128], [0, VROW], [1, 128]],
                )
                nc.vector.scalar_tensor_tensor(
                    out=acc3, in0=acc3, scalar=0.0, in1=w2ap,
                    op0=OP.max, op1=OP.mult,
                )
                nc.vector.tensor_reduce(
                    out=logit[:, VROW * ck : VROW * (ck + 1)].rearrange(
                        "p (r one) -> p r one", one=1
                    ),
                    in_=acc3,
                    axis=mybir.AxisListType.X,
                    op=OP.add,
                )
            ostg = lg.tile([128, Q], f32, tag="ostg", name=f"os{i}")
            nc.scalar.activation(ostg[:], logit[:], ACT.Sigmoid, bias=cb2_t[:])
            o_i = out_d.ap()[i]
            oap = AP(o_i.tensor, o_i.offset, [[1, 128], [128, Q]])
            nc.scalar.dma_start(oap, ostg[:])

    nc.compile()
    _CACHE["nc"] = nc
    return nc


def _host_prep(inputs):
    feat = np.asarray(inputs["feat_map"], dtype=np.float32)
    uv = np.asarray(inputs["verts_uv"], dtype=np.float32)
    rw = np.asarray(inputs["reduce_w"], dtype=np.float32)
    rb = np.asarray(inputs["reduce_b"], dtype=np.float32)
    w1 = np.asarray(inputs["cls_w1"], dtype=np.float32)
    b1 = np.asarray(inputs["cls_b1"], dtype=np.float32)
    w2 = np.asarray(inputs["cls_w2"], dtype=np.float32)
    b2 = np.asarray(inputs["cls_b2"], dtype=np.float32)

    rwt = np.ascontiguousarray(rw.T)                      # (256, 1280)
    uvp = np.zeros((B, NV, 2), dtype=np.float32)
    uvp[:, :N, :] = uv
    featr = feat.reshape(B, C, PIX).astype(ml_dtypes.bfloat16)

    shared = {
        "rwt": rwt,
        "cw1": np.ascontiguousarray(w1),
        "rb": rb,
        "cb1": b1,
        "w2r": np.ascontiguousarray(np.tile(w2[None, :], (128, 1))),
        "cb2": np.full((128, 1), b2[0], dtype=np.float32),
        "ident": np.eye(128, dtype=ml_dtypes.bfloat16),
    }
    in_maps = []
    for core in range(NCORES):
        sl = slice(core * IMGS, (core + 1) * IMGS)
        m = dict(shared)
        m["feat"] = np.ascontiguousarray(featr[sl])
        m["uv"] = np.ascontiguousarray(uvp[sl])
        in_maps.append(m)
    return in_maps


def kernel(**inputs):
    from concourse.bass_utils import run_bass_kernel_spmd

    nc = _build()
    in_maps = _host_prep(inputs)
    res = run_bass_kernel_spmd(nc, in_maps, list(range(NCORES)))
    out = np.empty((B, N), dtype=np.float32)
    for core in range(NCORES):
        dev = res.results[core]["out"]          # (IMGS, NV), vert j at col j
        out[core * IMGS : (core + 1) * IMGS] = dev[:, :N]
    return out
